# revision 6
# baseline (speedup 1.0000x reference)
"""Trainium2 Bass kernel for nn_BoundingBoxDiscipline.

Computes PENALTY_WEIGHT * mean_B(area_penalty + center_offset) where the
penalties are derived from per-sample bounding boxes of thresholded masks:
    pred_mask = max_C(prediction_probs) > 0.3
    true_mask = max_C(expected_onehot)  > 0.5

The bounding box of a [H, W] mask only needs two tiny reductions:
    row_any[y] = any_{x,c} (v[y,x,c] > T)
    col_any[x] = any_{y,c} (v[y,x,c] > T)
so the device reduces 512 MiB of input down to 1 KiB of row/col summaries
per sample and the exact bbox/penalty math happens on the host (all
comparisons are exact: v > T <=> relu(v - T) > 0 in fp32, and sums of
non-negative values are > 0 iff any element is > 0).

Device plan, data-parallel over batch (2 samples per core, 8 cores):
  - DMA: 16 tiles of [128 y-rows, 8192 (x*16+c)] fp32 per core (64 MiB).
  - ScalarE: relu(v - T) -> bf16 tile, with accum_out giving the per-row
    (free-axis) sum in one pass -> row_any.
  - TensorE: ones[128,1].T @ relu_tile[:, :, c] accumulated over the 4
    y-tiles and 16 channel views into one PSUM [1, 512] -> col sums.
"""

import numpy as np

import concourse.bacc as bacc
import concourse.tile as tile
from concourse import mybir
from concourse.bass_utils import run_bass_kernel_spmd

N_CORES = 8
B, H, W, C = 16, 512, 512, 16
SAMPLES_PER_CORE = B // N_CORES          # 2
TILES_PER_SAMPLE = H // 128              # 4
FREE = W * C                             # 8192
THRESHOLDS = (0.3, 0.5)                  # (prediction_probs, expected_onehot)
PENALTY_WEIGHT = 0.05

f32 = mybir.dt.float32
bf16 = mybir.dt.bfloat16


def build_nc(repeat: int = 1):
    """Build the per-core Bass module. `repeat` wraps the body in a device
    loop (used only for wall-clock timing; the graded path uses repeat=1)."""
    nc = bacc.Bacc("TRN2", debug=False)

    n_tiles = 2 * SAMPLES_PER_CORE * TILES_PER_SAMPLE  # 16 (tensor, sample, ytile)
    n_st = 2 * SAMPLES_PER_CORE                        # 4 sample-tensors

    pred = nc.dram_tensor(
        "pred", [SAMPLES_PER_CORE * TILES_PER_SAMPLE, 128, FREE], f32,
        kind="ExternalInput").ap()
    exp = nc.dram_tensor(
        "exp", [SAMPLES_PER_CORE * TILES_PER_SAMPLE, 128, FREE], f32,
        kind="ExternalInput").ap()
    rows = nc.dram_tensor("rows", [128, n_tiles], f32, kind="ExternalOutput").ap()
    cols = nc.dram_tensor("cols", [n_st, W], f32, kind="ExternalOutput").ap()

    with tile.TileContext(nc) as tc:
        with (
            tc.tile_pool(name="singles", bufs=1) as singles,
            tc.tile_pool(name="loads", bufs=3) as loads,
            tc.tile_pool(name="relus", bufs=3) as relus,
            tc.tile_pool(name="rowsp", bufs=1) as rowsp,
            tc.tile_pool(name="colsb", bufs=2) as colsb,
            tc.tile_pool(name="psum", bufs=2, space="PSUM") as psum,
        ):
            ones = singles.tile([128, 1], bf16)
            nc.vector.memset(ones, 1.0)
            biases = []
            for thr in THRESHOLDS:
                bias_t = singles.tile([128, 1], f32, tag=f"bias{thr}")
                nc.vector.memset(bias_t, -thr)
                biases.append(bias_t)
            rows_sb = rowsp.tile([128, n_tiles], f32)

            def body(_iv=None):
                # The very last tile is processed in 4 free-dim chunks so the
                # trailing (post-final-DMA) compute is ~2 us instead of ~13 us.
                N_CHUNKS = 4
                CHUNK = FREE // N_CHUNKS          # 2048 free elems = 128 x's
                XC = CHUNK // C                   # 128
                last_scratch = rowsp.tile([128, N_CHUNKS], f32)

                for tensor_idx, src in ((0, pred), (1, exp)):
                    bias_t = biases[tensor_idx]
                    for s in range(SAMPLES_PER_CORE):
                        st = tensor_idx * SAMPLES_PER_CORE + s
                        is_last_st = st == 2 * SAMPLES_PER_CORE - 1
                        psum_t = psum.tile([1, W], f32)
                        for t in range(TILES_PER_SAMPLE):
                            k = tensor_idx * 8 + s * 4 + t
                            if is_last_st and t == TILES_PER_SAMPLE - 1:
                                for ch in range(N_CHUNKS):
                                    ldc = chunkp.tile([128, CHUNK], f32, tag="ldc")
                                    nc.sync.dma_start(
                                        out=ldc,
                                        in_=src[s * 4 + t, :,
                                                ch * CHUNK:(ch + 1) * CHUNK])
                                    rlc = chunkp.tile([128, CHUNK], bf16, tag="rlc")
                                    nc.scalar.activation(
                                        out=rlc, in_=ldc,
                                        func=mybir.ActivationFunctionType.Relu,
                                        bias=bias_t, scale=1.0,
                                        accum_out=last_scratch[:, ch : ch + 1],
                                    )
                                    rlc3 = rlc.rearrange("p (x c) -> p x c", c=C)
                                    for ci in range(C):
                                        nc.tensor.matmul(
                                            psum_t[:, ch * XC:(ch + 1) * XC],
                                            ones, rlc3[:, :, ci],
                                            start=False,
                                            stop=(ch == N_CHUNKS - 1 and ci == C - 1),
                                        )
                                nc.vector.reduce_max(
                                    rows_sb[:, k : k + 1], last_scratch,
                                    axis=mybir.AxisListType.X)
                            else:
                                ld = loads.tile([128, FREE], f32)
                                nc.sync.dma_start(out=ld, in_=src[s * 4 + t])
                                rl = relus.tile([128, FREE], bf16)
                                nc.scalar.activation(
                                    out=rl, in_=ld,
                                    func=mybir.ActivationFunctionType.Relu,
                                    bias=bias_t, scale=1.0,
                                    accum_out=rows_sb[:, k : k + 1],
                                )
                                rl3 = rl.rearrange("p (x c) -> p x c", c=C)
                                for ci in range(C):
                                    nc.tensor.matmul(
                                        psum_t, ones, rl3[:, :, ci],
                                        start=(t == 0 and ci == 0),
                                        stop=False,
                                    )
                        csb = colsb.tile([1, W], f32)
                        nc.vector.tensor_copy(csb, psum_t)
                        nc.sync.dma_start(out=cols[st : st + 1], in_=csb)
                nc.sync.dma_start(out=rows, in_=rows_sb)

            if repeat == 1:
                body()
            else:
                with tc.For_i(0, repeat, 1,
                              hint_engines=(mybir.EngineType.PE,)) as iv:
                    body(iv)

    nc.compile()
    return nc


def _shard_inputs(prediction_probs, expected_onehot):
    p = np.ascontiguousarray(np.asarray(prediction_probs), dtype=np.float32)
    e = np.ascontiguousarray(np.asarray(expected_onehot), dtype=np.float32)
    p = p.reshape(N_CORES, SAMPLES_PER_CORE * TILES_PER_SAMPLE, 128, FREE)
    e = e.reshape(N_CORES, SAMPLES_PER_CORE * TILES_PER_SAMPLE, 128, FREE)
    return [{"pred": p[c], "exp": e[c]} for c in range(N_CORES)]


def _bbox_from_any(row_any, col_any):
    ys = np.nonzero(row_any)[0]
    xs = np.nonzero(col_any)[0]
    if ys.size == 0:
        return 0, 0, 1, 1
    return int(ys[0]), int(xs[0]), int(ys[-1]), int(xs[-1])


def _combine(results):
    """Host epilogue: exact bbox/penalty math from row/col summaries."""
    f = np.float32
    penalties = []
    for core in range(N_CORES):
        rows = results[core]["rows"]  # [128, 16]
        cols = results[core]["cols"]  # [4, 512]
        for s in range(SAMPLES_PER_CORE):
            boxes = []
            for tensor_idx in range(2):
                k0 = tensor_idx * 8 + s * 4
                row_any = rows[:, k0 : k0 + 4].T.ravel() > 0  # y = t*128 + p
                col_any = cols[tensor_idx * SAMPLES_PER_CORE + s] > 0
                boxes.append(_bbox_from_any(row_any, col_any))
            (py1, px1, py2, px2), (ty1, tx1, ty2, tx2) = boxes
            pred_area = f((py2 - py1 + 1) * (px2 - px1 + 1))
            true_area = f((ty2 - ty1 + 1) * (tx2 - tx1 + 1))
            area_penalty = f(max(f(0.0), f(pred_area - true_area))) / f(true_area + f(1.0))
            pcy, pcx = f(py1 + py2) / f(2.0), f(px1 + px2) / f(2.0)
            tcy, tcx = f(ty1 + ty2) / f(2.0), f(tx1 + tx2) / f(2.0)
            center_offset = np.sqrt(np.square(f(pcy - tcy)) + np.square(f(pcx - tcx))) / f(20.0)
            penalties.append(f(area_penalty + center_offset))
    mean = np.mean(np.asarray(penalties, dtype=np.float32), dtype=np.float32)
    return np.asarray(np.float32(PENALTY_WEIGHT) * mean, dtype=np.float32)


_NC_CACHE = {}


def kernel(prediction_probs, expected_onehot):
    if "nc" not in _NC_CACHE:
        _NC_CACHE["nc"] = build_nc()
    nc = _NC_CACHE["nc"]
    in_maps = _shard_inputs(prediction_probs, expected_onehot)
    res = run_bass_kernel_spmd(nc, in_maps, core_ids=list(range(N_CORES)))
    return _combine(res.results)


# revision 19
# speedup vs baseline: 1.3393x; 1.3393x over previous
"""Trainium2 Bass kernel for nn_BoundingBoxDiscipline.

Computes PENALTY_WEIGHT * mean_B(area_penalty + center_offset) where the
penalties are derived from per-sample bounding boxes of thresholded masks:
    pred_mask = max_C(prediction_probs) > 0.3
    true_mask = max_C(expected_onehot)  > 0.5

The bounding box of a [H, W] mask only needs two tiny reductions:
    row_any[y] = any_{x,c} (v[y,x,c] > T)
    col_any[x] = any_{y,c} (v[y,x,c] > T)
so the device reduces 512 MiB of input down to 1 KiB of row/col summaries
per sample and the exact bbox/penalty math happens on the host (all
comparisons are exact: v > T <=> relu(v - T) > 0 in fp32, and sums of
non-negative values are > 0 iff any element is > 0).

Device plan, data-parallel over batch (2 samples per core, 8 cores):
  - DMA (the roofline: 64 MiB/core at ~342 GB/s measured): 16 tiles of
    [128 y-rows, 8192 (x*16+c)] fp32 per core.
  - ScalarE: relu(v - T) -> bf16 tile, with accum_out giving the per-row
    (free-axis) sum in one pass -> row_any.
  - VectorE: pairwise max folds channels 16 -> 4 (positivity-preserving),
    keeping the cold-clocked TensorE off the critical path.
  - TensorE: ones[128,1].T @ folded[:, :, c] accumulated over 4 y-tiles
    and 4 channel views into one PSUM [1, 512] -> col sums.
The last tile is split into 4 chunks to shrink the post-final-DMA tail.
Measured 205.8 us/iter steady-state vs a 196.2 us DMA-only floor.
"""

import time

import numpy as np

import concourse.bacc as bacc
import concourse.tile as tile
from concourse import mybir
from concourse.bass_utils import run_bass_kernel_spmd

N_CORES = 8
B, H, W, C = 16, 512, 512, 16
SAMPLES_PER_CORE = B // N_CORES          # 2
TILES_PER_SAMPLE = H // 128              # 4
FREE = W * C                             # 8192
THRESHOLDS = (0.3, 0.5)                  # (prediction_probs, expected_onehot)
PENALTY_WEIGHT = 0.05

f32 = mybir.dt.float32
bf16 = mybir.dt.bfloat16


def build_nc(repeat: int = 1, do_act: bool = True, do_mm: bool = True,
             c_fold: int = 4, dma_alt: bool = False):
    """Build the per-core Bass module. `repeat` wraps the body in a device
    loop; `do_act`/`do_mm` exist only for ablation timing experiments.
    `c_fold` is the channel count after DVE pairwise max-folding (16 = no
    fold); folding moves colsum work off the cold-clocked TensorE onto the
    otherwise-idle DVE. The graded path uses the defaults."""
    assert c_fold in (2, 4, 8, 16)
    nc = bacc.Bacc("TRN2", debug=False)

    n_tiles = 2 * SAMPLES_PER_CORE * TILES_PER_SAMPLE  # 16 (tensor, sample, ytile)
    n_st = 2 * SAMPLES_PER_CORE                        # 4 sample-tensors

    pred = nc.dram_tensor(
        "pred", [SAMPLES_PER_CORE * TILES_PER_SAMPLE, 128, FREE], f32,
        kind="ExternalInput").ap()
    exp = nc.dram_tensor(
        "exp", [SAMPLES_PER_CORE * TILES_PER_SAMPLE, 128, FREE], f32,
        kind="ExternalInput").ap()
    rows = nc.dram_tensor("rows", [128, n_tiles], f32, kind="ExternalOutput").ap()
    cols = nc.dram_tensor("cols", [n_st, W], f32, kind="ExternalOutput").ap()

    with tile.TileContext(nc) as tc:
        with (
            tc.tile_pool(name="singles", bufs=1) as singles,
            tc.tile_pool(name="loads", bufs=3) as loads,
            tc.tile_pool(name="relus", bufs=2) as relus,
            tc.tile_pool(name="rowsp", bufs=1) as rowsp,
            tc.tile_pool(name="chunkp", bufs=2) as chunkp,
            tc.tile_pool(name="colsb", bufs=2) as colsb,
            tc.tile_pool(name="psum", bufs=2, space="PSUM") as psum,
        ):
            ones = singles.tile([128, 1], bf16)
            nc.vector.memset(ones, 1.0)
            biases = []
            for thr in THRESHOLDS:
                bias_t = singles.tile([128, 1], f32, tag=f"bias{thr}")
                nc.vector.memset(bias_t, -thr)
                biases.append(bias_t)
            rows_sb = rowsp.tile([128, n_tiles], f32)

            def fold_c(rl_flat, n_x, tag):
                """DVE pairwise max over channel halves: 16 -> c_fold chans.
                Positivity-preserving, so col_any is unchanged."""
                cur, cur_c = rl_flat, C
                while cur_c > c_fold:
                    nxt = cur_c // 2
                    out_t = relus.tile([128, n_x * nxt], bf16, tag=f"{tag}{nxt}")
                    cur3 = cur.rearrange("p (x c) -> p x c", c=cur_c)
                    out3 = out_t.rearrange("p (x c) -> p x c", c=nxt)
                    nc.vector.tensor_tensor(
                        out=out3, in0=cur3[:, :, 0:nxt], in1=cur3[:, :, nxt:cur_c],
                        op=mybir.AluOpType.max)
                    cur, cur_c = out_t, nxt
                return cur.rearrange("p (x c) -> p x c", c=cur_c), cur_c

            def body(_iv=None):
                # The very last tile is processed in 4 free-dim chunks so the
                # trailing (post-final-DMA) compute is ~2 us instead of ~13 us.
                N_CHUNKS = 4
                CHUNK = FREE // N_CHUNKS          # 2048 free elems = 128 x's
                XC = CHUNK // C                   # 128
                last_scratch = rowsp.tile([128, N_CHUNKS], f32)

                for tensor_idx, src in ((0, pred), (1, exp)):
                    bias_t = biases[tensor_idx]
                    for s in range(SAMPLES_PER_CORE):
                        st = tensor_idx * SAMPLES_PER_CORE + s
                        is_last_st = st == 2 * SAMPLES_PER_CORE - 1
                        psum_t = psum.tile([1, W], f32)
                        for t in range(TILES_PER_SAMPLE):
                            k = tensor_idx * 8 + s * 4 + t
                            if is_last_st and t == TILES_PER_SAMPLE - 1:
                                for ch in range(N_CHUNKS):
                                    ldc = chunkp.tile([128, CHUNK], f32, tag="ldc")
                                    nc.sync.dma_start(
                                        out=ldc,
                                        in_=src[s * 4 + t, :,
                                                ch * CHUNK:(ch + 1) * CHUNK])
                                    rlc = chunkp.tile([128, CHUNK], bf16, tag="rlc")
                                    if do_act:
                                        nc.scalar.activation(
                                            out=rlc, in_=ldc,
                                            func=mybir.ActivationFunctionType.Relu,
                                            bias=bias_t, scale=1.0,
                                            accum_out=last_scratch[:, ch : ch + 1],
                                        )
                                    if do_mm:
                                        rlc3, n_c = fold_c(rlc, XC, "foldc")
                                        for ci in range(n_c):
                                            nc.tensor.matmul(
                                                psum_t[:, ch * XC:(ch + 1) * XC],
                                                ones, rlc3[:, :, ci],
                                                start=False,
                                                stop=(ch == N_CHUNKS - 1 and ci == n_c - 1),
                                            )
                                if do_act:
                                    nc.vector.reduce_max(
                                        rows_sb[:, k : k + 1], last_scratch,
                                        axis=mybir.AxisListType.X)
                            else:
                                ld = loads.tile([128, FREE], f32)
                                dma_eng = (nc.scalar if (dma_alt and (k % 2)) else nc.sync)
                                dma_eng.dma_start(out=ld, in_=src[s * 4 + t])
                                rl = relus.tile([128, FREE], bf16)
                                if do_act:
                                    nc.scalar.activation(
                                        out=rl, in_=ld,
                                        func=mybir.ActivationFunctionType.Relu,
                                        bias=bias_t, scale=1.0,
                                        accum_out=rows_sb[:, k : k + 1],
                                    )
                                if do_mm:
                                    rl3, n_c = fold_c(rl, W, "fold")
                                    for ci in range(n_c):
                                        nc.tensor.matmul(
                                            psum_t, ones, rl3[:, :, ci],
                                            start=(t == 0 and ci == 0),
                                            stop=(not is_last_st
                                                  and t == TILES_PER_SAMPLE - 1
                                                  and ci == n_c - 1),
                                        )
                        if do_mm:
                            csb = colsb.tile([1, W], f32)
                            nc.vector.tensor_copy(csb, psum_t)
                            nc.sync.dma_start(out=cols[st : st + 1], in_=csb)
                if not do_mm:
                    csb = colsb.tile([4, W], f32)
                    nc.vector.memset(csb, 1.0)
                    nc.sync.dma_start(out=cols, in_=csb)
                if not do_act:
                    nc.vector.memset(rows_sb[:, :1], 1.0)
                nc.sync.dma_start(out=rows, in_=rows_sb)

            if repeat == 1:
                body()
            else:
                with tc.For_i(0, repeat, 1,
                              hint_engines=(mybir.EngineType.PE,)) as iv:
                    body(iv)

    nc.compile()
    return nc


def _shard_inputs(prediction_probs, expected_onehot):
    p = np.ascontiguousarray(np.asarray(prediction_probs), dtype=np.float32)
    e = np.ascontiguousarray(np.asarray(expected_onehot), dtype=np.float32)
    p = p.reshape(N_CORES, SAMPLES_PER_CORE * TILES_PER_SAMPLE, 128, FREE)
    e = e.reshape(N_CORES, SAMPLES_PER_CORE * TILES_PER_SAMPLE, 128, FREE)
    return [{"pred": p[c], "exp": e[c]} for c in range(N_CORES)]


def _bbox_from_any(row_any, col_any):
    ys = np.nonzero(row_any)[0]
    xs = np.nonzero(col_any)[0]
    if ys.size == 0:
        return 0, 0, 1, 1
    return int(ys[0]), int(xs[0]), int(ys[-1]), int(xs[-1])


def _combine(results):
    """Host epilogue: exact bbox/penalty math from row/col summaries."""
    f = np.float32
    penalties = []
    for core in range(N_CORES):
        rows = results[core]["rows"]  # [128, 16]
        cols = results[core]["cols"]  # [4, 512]
        for s in range(SAMPLES_PER_CORE):
            boxes = []
            for tensor_idx in range(2):
                k0 = tensor_idx * 8 + s * 4
                row_any = rows[:, k0 : k0 + 4].T.ravel() > 0  # y = t*128 + p
                col_any = cols[tensor_idx * SAMPLES_PER_CORE + s] > 0
                boxes.append(_bbox_from_any(row_any, col_any))
            (py1, px1, py2, px2), (ty1, tx1, ty2, tx2) = boxes
            pred_area = f((py2 - py1 + 1) * (px2 - px1 + 1))
            true_area = f((ty2 - ty1 + 1) * (tx2 - tx1 + 1))
            area_penalty = f(max(f(0.0), f(pred_area - true_area))) / f(true_area + f(1.0))
            pcy, pcx = f(py1 + py2) / f(2.0), f(px1 + px2) / f(2.0)
            tcy, tcx = f(ty1 + ty2) / f(2.0), f(tx1 + tx2) / f(2.0)
            center_offset = np.sqrt(np.square(f(pcy - tcy)) + np.square(f(pcx - tcx))) / f(20.0)
            penalties.append(f(area_penalty + center_offset))
    mean = np.mean(np.asarray(penalties, dtype=np.float32), dtype=np.float32)
    return np.asarray(np.float32(PENALTY_WEIGHT) * mean, dtype=np.float32)


_NC_CACHE = {}


def kernel(prediction_probs, expected_onehot):
    if "nc" not in _NC_CACHE:
        _NC_CACHE["nc"] = build_nc()
    nc = _NC_CACHE["nc"]
    in_maps = _shard_inputs(prediction_probs, expected_onehot)
    last_exc = None
    for attempt in range(3):  # the axon device occasionally flakes transiently
        try:
            res = run_bass_kernel_spmd(nc, in_maps, core_ids=list(range(N_CORES)))
            return _combine(res.results)
        except Exception as e:  # noqa: BLE001
            last_exc = e
            time.sleep(5.0)
    raise last_exc


# revision 22
# speedup vs baseline: 1.3454x; 1.0046x over previous
"""Trainium2 Bass kernel for nn_BoundingBoxDiscipline.

Computes PENALTY_WEIGHT * mean_B(area_penalty + center_offset) where the
penalties are derived from per-sample bounding boxes of thresholded masks:
    pred_mask = max_C(prediction_probs) > 0.3
    true_mask = max_C(expected_onehot)  > 0.5

The bounding box of a [H, W] mask only needs two tiny reductions:
    row_any[y] = any_{x,c} (v[y,x,c] > T)
    col_any[x] = any_{y,c} (v[y,x,c] > T)
so the device reduces 512 MiB of input down to 1 KiB of row/col summaries
per sample and the exact bbox/penalty math happens on the host (all
comparisons are exact: v > T <=> relu(v - T) > 0 in fp32, and sums of
non-negative values are > 0 iff any element is > 0).

Device plan, data-parallel over batch (2 samples per core, 8 cores):
  - DMA (the roofline: 64 MiB/core at ~342 GB/s measured): 16 tiles of
    [128 y-rows, 8192 (x*16+c)] fp32 per core.
  - ScalarE: relu(v - T) -> bf16 tile, with accum_out giving the per-row
    (free-axis) sum in one pass -> row_any.
  - VectorE: pairwise max folds channels 16 -> 4 (positivity-preserving),
    keeping the cold-clocked TensorE off the critical path.
  - TensorE: ones[128,1].T @ folded[:, :, c] accumulated over 4 y-tiles
    and 4 channel views into one PSUM [1, 512] -> col sums.
The last tile is split into 4 chunks to shrink the post-final-DMA tail.
Measured 205.8 us/iter steady-state vs a 196.2 us DMA-only floor.
"""

import time

import numpy as np

import concourse.bacc as bacc
import concourse.tile as tile
from concourse import mybir
from concourse.bass_utils import run_bass_kernel_spmd

N_CORES = 8
B, H, W, C = 16, 512, 512, 16
SAMPLES_PER_CORE = B // N_CORES          # 2
TILES_PER_SAMPLE = H // 128              # 4
FREE = W * C                             # 8192
THRESHOLDS = (0.3, 0.5)                  # (prediction_probs, expected_onehot)
PENALTY_WEIGHT = 0.05

f32 = mybir.dt.float32
bf16 = mybir.dt.bfloat16


def build_nc(repeat: int = 1, do_act: bool = True, do_mm: bool = True,
             c_fold: int = 4, dma_alt: bool = False):
    """Build the per-core Bass module. `repeat` wraps the body in a device
    loop; `do_act`/`do_mm` exist only for ablation timing experiments.
    `c_fold` is the channel count after DVE pairwise max-folding (16 = no
    fold); folding moves colsum work off the cold-clocked TensorE onto the
    otherwise-idle DVE. The graded path uses the defaults."""
    assert c_fold in (2, 4, 8, 16)
    nc = bacc.Bacc("TRN2", debug=False)

    n_tiles = 2 * SAMPLES_PER_CORE * TILES_PER_SAMPLE  # 16 (tensor, sample, ytile)
    n_st = 2 * SAMPLES_PER_CORE                        # 4 sample-tensors

    pred = nc.dram_tensor(
        "pred", [SAMPLES_PER_CORE * TILES_PER_SAMPLE, 128, FREE], f32,
        kind="ExternalInput").ap()
    exp = nc.dram_tensor(
        "exp", [SAMPLES_PER_CORE * TILES_PER_SAMPLE, 128, FREE], f32,
        kind="ExternalInput").ap()
    rows = nc.dram_tensor("rows", [128, n_tiles], f32, kind="ExternalOutput").ap()
    cols = nc.dram_tensor("cols", [n_st, W], f32, kind="ExternalOutput").ap()

    with tile.TileContext(nc) as tc:
        with (
            tc.tile_pool(name="singles", bufs=1) as singles,
            tc.tile_pool(name="loads", bufs=3) as loads,
            tc.tile_pool(name="relus", bufs=2) as relus,
            tc.tile_pool(name="rowsp", bufs=1) as rowsp,
            tc.tile_pool(name="chunkp", bufs=2) as chunkp,
            tc.tile_pool(name="colsb", bufs=2) as colsb,
            tc.tile_pool(name="psum", bufs=2, space="PSUM") as psum,
        ):
            ones = singles.tile([128, 1], bf16)
            nc.vector.memset(ones, 1.0)
            biases = []
            for thr in THRESHOLDS:
                bias_t = singles.tile([128, 1], f32, tag=f"bias{thr}")
                nc.vector.memset(bias_t, -thr)
                biases.append(bias_t)
            rows_sb = rowsp.tile([128, n_tiles], f32)

            def fold_c(rl_flat, n_x, tag):
                """DVE pairwise max over channel halves: 16 -> c_fold chans.
                Positivity-preserving, so col_any is unchanged."""
                cur, cur_c = rl_flat, C
                while cur_c > c_fold:
                    nxt = cur_c // 2
                    out_t = relus.tile([128, n_x * nxt], bf16, tag=f"{tag}{nxt}")
                    cur3 = cur.rearrange("p (x c) -> p x c", c=cur_c)
                    out3 = out_t.rearrange("p (x c) -> p x c", c=nxt)
                    nc.vector.tensor_tensor(
                        out=out3, in0=cur3[:, :, 0:nxt], in1=cur3[:, :, nxt:cur_c],
                        op=mybir.AluOpType.max)
                    cur, cur_c = out_t, nxt
                return cur.rearrange("p (x c) -> p x c", c=cur_c), cur_c

            def body(_iv=None):
                # The very last tile is processed in 4 free-dim chunks so the
                # trailing (post-final-DMA) compute is ~2 us instead of ~13 us.
                N_CHUNKS = 4
                CHUNK = FREE // N_CHUNKS          # 2048 free elems = 128 x's
                XC = CHUNK // C                   # 128
                last_scratch = rowsp.tile([128, N_CHUNKS], f32)

                for tensor_idx, src in ((0, pred), (1, exp)):
                    bias_t = biases[tensor_idx]
                    for s in range(SAMPLES_PER_CORE):
                        st = tensor_idx * SAMPLES_PER_CORE + s
                        is_last_st = st == 2 * SAMPLES_PER_CORE - 1
                        psum_t = psum.tile([1, W], f32)
                        for t in range(TILES_PER_SAMPLE):
                            k = tensor_idx * 8 + s * 4 + t
                            if is_last_st and t == TILES_PER_SAMPLE - 1:
                                for ch in range(N_CHUNKS):
                                    ldc = chunkp.tile([128, CHUNK], f32, tag="ldc")
                                    nc.sync.dma_start(
                                        out=ldc,
                                        in_=src[s * 4 + t, :,
                                                ch * CHUNK:(ch + 1) * CHUNK])
                                    rlc = chunkp.tile([128, CHUNK], bf16, tag="rlc")
                                    if do_act:
                                        nc.scalar.activation(
                                            out=rlc, in_=ldc,
                                            func=mybir.ActivationFunctionType.Relu,
                                            bias=bias_t, scale=1.0,
                                            accum_out=last_scratch[:, ch : ch + 1],
                                        )
                                    if do_mm:
                                        rlc3, n_c = fold_c(rlc, XC, "foldc")
                                        for ci in range(n_c):
                                            nc.tensor.matmul(
                                                psum_t[:, ch * XC:(ch + 1) * XC],
                                                ones, rlc3[:, :, ci],
                                                start=False,
                                                stop=(ch == N_CHUNKS - 1 and ci == n_c - 1),
                                            )
                                if do_act:
                                    nc.vector.reduce_max(
                                        rows_sb[:, k : k + 1], last_scratch,
                                        axis=mybir.AxisListType.X)
                            else:
                                ld = loads.tile([128, FREE], f32)
                                dma_eng = (nc.scalar if (dma_alt and (k % 2)) else nc.sync)
                                dma_eng.dma_start(out=ld, in_=src[s * 4 + t])
                                rl = relus.tile([128, FREE], bf16)
                                if do_act:
                                    nc.scalar.activation(
                                        out=rl, in_=ld,
                                        func=mybir.ActivationFunctionType.Relu,
                                        bias=bias_t, scale=1.0,
                                        accum_out=rows_sb[:, k : k + 1],
                                    )
                                if do_mm:
                                    rl3, n_c = fold_c(rl, W, "fold")
                                    for ci in range(n_c):
                                        nc.tensor.matmul(
                                            psum_t, ones, rl3[:, :, ci],
                                            start=(t == 0 and ci == 0),
                                            stop=(not is_last_st
                                                  and t == TILES_PER_SAMPLE - 1
                                                  and ci == n_c - 1),
                                        )
                        if do_mm:
                            csb = colsb.tile([1, W], f32)
                            nc.vector.tensor_copy(csb, psum_t)
                            nc.sync.dma_start(out=cols[st : st + 1], in_=csb)
                if not do_mm:
                    csb = colsb.tile([4, W], f32)
                    nc.vector.memset(csb, 1.0)
                    nc.sync.dma_start(out=cols, in_=csb)
                if not do_act:
                    nc.vector.memset(rows_sb[:, :1], 1.0)
                nc.sync.dma_start(out=rows, in_=rows_sb)

            if repeat == 1:
                body()
            else:
                with tc.For_i(0, repeat, 1,
                              hint_engines=(mybir.EngineType.PE,)) as iv:
                    body(iv)

    nc.compile()
    return nc


def _shard_inputs(prediction_probs, expected_onehot):
    p = np.ascontiguousarray(np.asarray(prediction_probs), dtype=np.float32)
    e = np.ascontiguousarray(np.asarray(expected_onehot), dtype=np.float32)
    p = p.reshape(N_CORES, SAMPLES_PER_CORE * TILES_PER_SAMPLE, 128, FREE)
    e = e.reshape(N_CORES, SAMPLES_PER_CORE * TILES_PER_SAMPLE, 128, FREE)
    return [{"pred": p[c], "exp": e[c]} for c in range(N_CORES)]


def _bbox_from_any(row_any, col_any):
    ys = np.nonzero(row_any)[0]
    xs = np.nonzero(col_any)[0]
    if ys.size == 0:
        return 0, 0, 1, 1
    return int(ys[0]), int(xs[0]), int(ys[-1]), int(xs[-1])


def _combine(results):
    """Host epilogue: exact bbox/penalty math from row/col summaries."""
    f = np.float32
    penalties = []
    for core in range(N_CORES):
        rows = results[core]["rows"]  # [128, 16]
        cols = results[core]["cols"]  # [4, 512]
        for s in range(SAMPLES_PER_CORE):
            boxes = []
            for tensor_idx in range(2):
                k0 = tensor_idx * 8 + s * 4
                row_any = rows[:, k0 : k0 + 4].T.ravel() > 0  # y = t*128 + p
                col_any = cols[tensor_idx * SAMPLES_PER_CORE + s] > 0
                boxes.append(_bbox_from_any(row_any, col_any))
            (py1, px1, py2, px2), (ty1, tx1, ty2, tx2) = boxes
            pred_area = f((py2 - py1 + 1) * (px2 - px1 + 1))
            true_area = f((ty2 - ty1 + 1) * (tx2 - tx1 + 1))
            area_penalty = f(max(f(0.0), f(pred_area - true_area))) / f(true_area + f(1.0))
            pcy, pcx = f(py1 + py2) / f(2.0), f(px1 + px2) / f(2.0)
            tcy, tcx = f(ty1 + ty2) / f(2.0), f(tx1 + tx2) / f(2.0)
            center_offset = np.sqrt(np.square(f(pcy - tcy)) + np.square(f(pcx - tcx))) / f(20.0)
            penalties.append(f(area_penalty + center_offset))
    mean = np.mean(np.asarray(penalties, dtype=np.float32), dtype=np.float32)
    return np.asarray(np.float32(PENALTY_WEIGHT) * mean, dtype=np.float32)


_NC_CACHE = {}


def kernel(prediction_probs, expected_onehot):
    if "nc" not in _NC_CACHE:
        _NC_CACHE["nc"] = build_nc()
    nc = _NC_CACHE["nc"]
    in_maps = _shard_inputs(prediction_probs, expected_onehot)
    last_exc = None
    for attempt in range(3):  # the axon device occasionally flakes transiently
        try:
            res = run_bass_kernel_spmd(nc, in_maps, core_ids=list(range(N_CORES)))
            return _combine(res.results)
        except Exception as e:  # noqa: BLE001
            last_exc = e
            try:
                # an NRT_EXEC_UNIT_UNRECOVERABLE poisons the PJRT mesh for
                # the whole process; dropping the backend forces a reconnect
                import jax.extend.backend

                jax.extend.backend.clear_backends()
            except Exception:  # noqa: BLE001
                pass
            time.sleep(5.0)
    raise last_exc


# revision 36
# speedup vs baseline: 1.3705x; 1.0187x over previous
"""Trainium2 Bass kernel for nn_BoundingBoxDiscipline.

Computes PENALTY_WEIGHT * mean_B(area_penalty + center_offset) where the
penalties are derived from per-sample bounding boxes of thresholded masks:
    pred_mask = max_C(prediction_probs) > 0.3
    true_mask = max_C(expected_onehot)  > 0.5

The bounding box of a [H, W] mask only needs two tiny reductions:
    row_any[y] = any_{x,c} (v[y,x,c] > T)
    col_any[x] = any_{y,c} (v[y,x,c] > T)
so the device reduces 512 MiB of input down to 1 KiB of row/col summaries
per sample and the exact bbox/penalty math happens on the host (all
comparisons are exact: v > T <=> relu(v - T) > 0 in fp32, and sums of
non-negative values are > 0 iff any element is > 0).

Device plan, data-parallel over batch (2 samples per core, 8 cores):
  - DMA (the roofline: 64 MiB/core at ~342 GB/s measured): 16 tiles of
    [128 y-rows, 8192 (x*16+c)] fp32 per core.
  - ScalarE: relu(v - T) -> bf16 tile, with accum_out giving the per-row
    (free-axis) sum in one pass -> row_any.
  - VectorE: pairwise max folds channels 16 -> 4 (positivity-preserving),
    keeping the cold-clocked TensorE off the critical path.
  - TensorE: ones[128,1].T @ folded[:, :, c] accumulated over 4 y-tiles
    and 4 channel views into one PSUM [1, 512] -> col sums.
The last two tiles are split into 8 chunks each (chunk pool bufs=8) so the
trailing ScalarE work drains while the final bytes stream in instead of
stalling the chunk DMAs. Measured 201.7 us/iter steady-state vs a 196.2 us
DMA-only floor (342 GB/s/core effective; 358 GB/s HBM spec).
"""

import time

import numpy as np

import concourse.bacc as bacc
import concourse.tile as tile
from concourse import mybir
from concourse.bass_utils import run_bass_kernel_spmd

N_CORES = 8
B, H, W, C = 16, 512, 512, 16
SAMPLES_PER_CORE = B // N_CORES          # 2
TILES_PER_SAMPLE = H // 128              # 4
FREE = W * C                             # 8192
THRESHOLDS = (0.3, 0.5)                  # (prediction_probs, expected_onehot)
PENALTY_WEIGHT = 0.05

f32 = mybir.dt.float32
bf16 = mybir.dt.bfloat16


def build_nc(repeat: int = 1, do_act: bool = True, do_mm: bool = True,
             c_fold: int = 4, dma_alt: bool = False):
    """Build the per-core Bass module. `repeat` wraps the body in a device
    loop; `do_act`/`do_mm` exist only for ablation timing experiments.
    `c_fold` is the channel count after DVE pairwise max-folding (16 = no
    fold); folding moves colsum work off the cold-clocked TensorE onto the
    otherwise-idle DVE. The graded path uses the defaults."""
    assert c_fold in (2, 4, 8, 16)
    nc = bacc.Bacc("TRN2", debug=False)

    n_tiles = 2 * SAMPLES_PER_CORE * TILES_PER_SAMPLE  # 16 (tensor, sample, ytile)
    n_st = 2 * SAMPLES_PER_CORE                        # 4 sample-tensors

    pred = nc.dram_tensor(
        "pred", [SAMPLES_PER_CORE * TILES_PER_SAMPLE, 128, FREE], f32,
        kind="ExternalInput").ap()
    exp = nc.dram_tensor(
        "exp", [SAMPLES_PER_CORE * TILES_PER_SAMPLE, 128, FREE], f32,
        kind="ExternalInput").ap()
    rows = nc.dram_tensor("rows", [128, n_tiles], f32, kind="ExternalOutput").ap()
    cols = nc.dram_tensor("cols", [n_st, W], f32, kind="ExternalOutput").ap()

    with tile.TileContext(nc) as tc:
        with (
            tc.tile_pool(name="singles", bufs=1) as singles,
            tc.tile_pool(name="loads", bufs=2) as loads,
            tc.tile_pool(name="relus", bufs=2) as relus,
            tc.tile_pool(name="rowsp", bufs=1) as rowsp,
            tc.tile_pool(name="chunkp", bufs=8) as chunkp,
            tc.tile_pool(name="colsb", bufs=2) as colsb,
            tc.tile_pool(name="psum", bufs=2, space="PSUM") as psum,
        ):
            ones = singles.tile([128, 1], bf16)
            nc.vector.memset(ones, 1.0)
            biases = []
            for thr in THRESHOLDS:
                bias_t = singles.tile([128, 1], f32, tag=f"bias{thr}")
                nc.vector.memset(bias_t, -thr)
                biases.append(bias_t)
            rows_sb = rowsp.tile([128, n_tiles], f32)

            def fold_c(rl_flat, n_x, tag):
                """DVE pairwise max over channel halves: 16 -> c_fold chans.
                Positivity-preserving, so col_any is unchanged."""
                cur, cur_c = rl_flat, C
                while cur_c > c_fold:
                    nxt = cur_c // 2
                    out_t = relus.tile([128, n_x * nxt], bf16, tag=f"{tag}{nxt}")
                    cur3 = cur.rearrange("p (x c) -> p x c", c=cur_c)
                    out3 = out_t.rearrange("p (x c) -> p x c", c=nxt)
                    nc.vector.tensor_tensor(
                        out=out3, in0=cur3[:, :, 0:nxt], in1=cur3[:, :, nxt:cur_c],
                        op=mybir.AluOpType.max)
                    cur, cur_c = out_t, nxt
                return cur.rearrange("p (x c) -> p x c", c=cur_c), cur_c

            def body(_iv=None):
                # The last two tiles are processed in 4 free-dim chunks each:
                # every trailing ACT op is then ~1.9 us < the 2.9 us chunk DMA
                # cadence, so ScalarE never backlogs the tail and the
                # post-final-DMA compute is ~2 us instead of ~13 us.
                N_CHUNKS = 8
                N_CHUNK_TILES = 2
                CHUNK = FREE // N_CHUNKS          # 1024 free elems = 64 x's
                XC = CHUNK // C                   # 128
                last_scratch = rowsp.tile([128, N_CHUNK_TILES * N_CHUNKS], f32)

                for tensor_idx, src in ((0, pred), (1, exp)):
                    bias_t = biases[tensor_idx]
                    for s in range(SAMPLES_PER_CORE):
                        st = tensor_idx * SAMPLES_PER_CORE + s
                        is_last_st = st == 2 * SAMPLES_PER_CORE - 1
                        psum_t = psum.tile([1, W], f32)
                        if is_last_st and N_CHUNK_TILES >= TILES_PER_SAMPLE and do_mm:
                            # all tiles chunked -> no N=512 start=True matmul
                            # to clear the bank; zero it and accumulate onto
                            # zeros (correct under any has_written semantics)
                            nc.vector.memset(psum_t, 0.0)
                        for t in range(TILES_PER_SAMPLE):
                            k = tensor_idx * 8 + s * 4 + t
                            if is_last_st and t >= TILES_PER_SAMPLE - N_CHUNK_TILES:
                                ct = t - (TILES_PER_SAMPLE - N_CHUNK_TILES)
                                scr = last_scratch[:, ct * N_CHUNKS:(ct + 1) * N_CHUNKS]
                                is_last_tile = t == TILES_PER_SAMPLE - 1
                                for ch in range(N_CHUNKS):
                                    ldc = chunkp.tile([128, CHUNK], f32, tag="ldc")
                                    nc.sync.dma_start(
                                        out=ldc,
                                        in_=src[s * 4 + t, :,
                                                ch * CHUNK:(ch + 1) * CHUNK])
                                    rlc = chunkp.tile([128, CHUNK], bf16, tag="rlc")
                                    if do_act:
                                        nc.scalar.activation(
                                            out=rlc, in_=ldc,
                                            func=mybir.ActivationFunctionType.Relu,
                                            bias=bias_t, scale=1.0,
                                            accum_out=scr[:, ch : ch + 1],
                                        )
                                    if do_mm:
                                        rlc3, n_c = fold_c(rlc, XC, "foldc")
                                        for ci in range(n_c):
                                            nc.tensor.matmul(
                                                psum_t[:, ch * XC:(ch + 1) * XC],
                                                ones, rlc3[:, :, ci],
                                                start=False,
                                                stop=(is_last_tile
                                                      and ch == N_CHUNKS - 1
                                                      and ci == n_c - 1),
                                                # accumulation onto the
                                                # memset-zeroed bank; the sim's
                                                # bank-granular group assert
                                                # can't express this
                                                skip_group_check=(
                                                    N_CHUNK_TILES
                                                    >= TILES_PER_SAMPLE),
                                            )
                                if do_act:
                                    nc.vector.reduce_max(
                                        rows_sb[:, k : k + 1], scr,
                                        axis=mybir.AxisListType.X)
                            else:
                                ld = loads.tile([128, FREE], f32)
                                dma_eng = (nc.scalar if (dma_alt and (k % 2)) else nc.sync)
                                dma_eng.dma_start(out=ld, in_=src[s * 4 + t])
                                rl = relus.tile([128, FREE], bf16)
                                if do_act:
                                    nc.scalar.activation(
                                        out=rl, in_=ld,
                                        func=mybir.ActivationFunctionType.Relu,
                                        bias=bias_t, scale=1.0,
                                        accum_out=rows_sb[:, k : k + 1],
                                    )
                                if do_mm:
                                    rl3, n_c = fold_c(rl, W, "fold")
                                    for ci in range(n_c):
                                        nc.tensor.matmul(
                                            psum_t, ones, rl3[:, :, ci],
                                            start=(t == 0 and ci == 0),
                                            stop=(not is_last_st
                                                  and t == TILES_PER_SAMPLE - 1
                                                  and ci == n_c - 1),
                                        )
                        if do_mm:
                            csb = colsb.tile([1, W], f32)
                            nc.vector.tensor_copy(csb, psum_t)
                            nc.sync.dma_start(out=cols[st : st + 1], in_=csb)
                if not do_mm:
                    csb = colsb.tile([4, W], f32)
                    nc.vector.memset(csb, 1.0)
                    nc.sync.dma_start(out=cols, in_=csb)
                if not do_act:
                    nc.vector.memset(rows_sb[:, :1], 1.0)
                nc.sync.dma_start(out=rows, in_=rows_sb)

            if repeat == 1:
                body()
            else:
                with tc.For_i(0, repeat, 1,
                              hint_engines=(mybir.EngineType.PE,)) as iv:
                    body(iv)

    nc.compile()
    return nc


def _shard_inputs(prediction_probs, expected_onehot):
    p = np.ascontiguousarray(np.asarray(prediction_probs), dtype=np.float32)
    e = np.ascontiguousarray(np.asarray(expected_onehot), dtype=np.float32)
    p = p.reshape(N_CORES, SAMPLES_PER_CORE * TILES_PER_SAMPLE, 128, FREE)
    e = e.reshape(N_CORES, SAMPLES_PER_CORE * TILES_PER_SAMPLE, 128, FREE)
    return [{"pred": p[c], "exp": e[c]} for c in range(N_CORES)]


def _bbox_from_any(row_any, col_any):
    ys = np.nonzero(row_any)[0]
    xs = np.nonzero(col_any)[0]
    if ys.size == 0:
        return 0, 0, 1, 1
    return int(ys[0]), int(xs[0]), int(ys[-1]), int(xs[-1])


def _combine(results):
    """Host epilogue: exact bbox/penalty math from row/col summaries."""
    f = np.float32
    penalties = []
    for core in range(N_CORES):
        rows = results[core]["rows"]  # [128, 16]
        cols = results[core]["cols"]  # [4, 512]
        for s in range(SAMPLES_PER_CORE):
            boxes = []
            for tensor_idx in range(2):
                k0 = tensor_idx * 8 + s * 4
                row_any = rows[:, k0 : k0 + 4].T.ravel() > 0  # y = t*128 + p
                col_any = cols[tensor_idx * SAMPLES_PER_CORE + s] > 0
                boxes.append(_bbox_from_any(row_any, col_any))
            (py1, px1, py2, px2), (ty1, tx1, ty2, tx2) = boxes
            pred_area = f((py2 - py1 + 1) * (px2 - px1 + 1))
            true_area = f((ty2 - ty1 + 1) * (tx2 - tx1 + 1))
            area_penalty = f(max(f(0.0), f(pred_area - true_area))) / f(true_area + f(1.0))
            pcy, pcx = f(py1 + py2) / f(2.0), f(px1 + px2) / f(2.0)
            tcy, tcx = f(ty1 + ty2) / f(2.0), f(tx1 + tx2) / f(2.0)
            center_offset = np.sqrt(np.square(f(pcy - tcy)) + np.square(f(pcx - tcx))) / f(20.0)
            penalties.append(f(area_penalty + center_offset))
    mean = np.mean(np.asarray(penalties, dtype=np.float32), dtype=np.float32)
    return np.asarray(np.float32(PENALTY_WEIGHT) * mean, dtype=np.float32)


_NC_CACHE = {}


def kernel(prediction_probs, expected_onehot):
    if "nc" not in _NC_CACHE:
        _NC_CACHE["nc"] = build_nc()
    nc = _NC_CACHE["nc"]
    in_maps = _shard_inputs(prediction_probs, expected_onehot)
    last_exc = None
    for attempt in range(3):  # the axon device occasionally flakes transiently
        try:
            res = run_bass_kernel_spmd(nc, in_maps, core_ids=list(range(N_CORES)))
            return _combine(res.results)
        except Exception as e:  # noqa: BLE001
            last_exc = e
            try:
                # an NRT_EXEC_UNIT_UNRECOVERABLE poisons the PJRT mesh for
                # the whole process; dropping the backend forces a reconnect
                import jax.extend.backend

                jax.extend.backend.clear_backends()
            except Exception:  # noqa: BLE001
                pass
            time.sleep(5.0)
    raise last_exc
